# revision 6
# baseline (speedup 1.0000x reference)
"""AFNO-style spectral MLP (nn_AFMM_57166014709855) as a Bass/Tile kernel on 8 TRN2 cores.

y = irfft2( cMLP(rfft2(x)) * rfft2(x) ), grouped complex MLP in frequency
domain. FFTs are DFT matmuls on the tensor engine. Data-parallel over batch:
core b handles x[b] (512,128,128); weights replicated; no collectives.

Per-core dataflow for each group g (d=128 channels), f-layout is kh-major
(f = kh*65 + kw) so S2->S3 pipelines at fine grain:
  S1 (contract h): per ch:   psum[w,130] = x_c^T @ [FHr|FHi]
  S2 (contract w): per kh:   psum[c,130] = Yr@[Fwr|Fwi] + Yi@[-Fwi|Fwr]
  S3 (contract c): 512-f:    H = relu(W1 (x) S + b1)
  S4 (contract c): 512-f:    O = W2 (x) H + b2;  gate G = O * S (complex)
  S5: PE-transpose G -> GT [kh, kw*128+c]
  S6 (contract kh), mirror:  M1..M4 [kw,65]; V-lo/hi = M1-+M2 / M3+-M4
       (cos/sin symmetry h <-> 128-h halves the matmul columns)
  S7 (contract kw), mirror:  N1,N2 [h,65]; y-lo/hi = N1+-N2 (w <-> 128-w)
Evacuations are spread across Act/DVE/Pool to keep the tensor engine the
critical path; S6/S7 batches of group g drain during phases of group g+1.
"""

import numpy as np

import concourse.bass as bass
import concourse.tile as tile
import concourse.mybir as mybir
from concourse import bacc
from concourse.bass_utils import run_bass_kernel_spmd

F32 = mybir.dt.float32
BF16 = mybir.dt.bfloat16
AF = mybir.ActivationFunctionType
ALU = mybir.AluOpType

B, C, H, W = 8, 512, 128, 128
G, D, KH, KW = 4, 128, 128, 65
NF = KH * KW  # 8320
NCORES = 8


def _consts():
    s = 1.0 / np.sqrt(128.0)
    k = np.arange(KW)
    n = np.arange(128)
    th_w = np.outer(n, k) * (2 * np.pi / W)          # [w, kw]
    Fwr = np.cos(th_w) * s
    Fwi = -np.sin(th_w) * s
    th_h = np.outer(n, n) * (2 * np.pi / H)          # [h, kh] (symmetric)
    FHr_T = np.cos(th_h) * s
    FHi_T = -np.sin(th_h) * s
    IHr_T = np.cos(th_h) * s                         # [kh, h]
    IHi_T = np.sin(th_h) * s
    a = np.full(KW, 2.0)
    a[0] = 1.0
    a[64] = 1.0
    th_c = np.outer(k, n) * (2 * np.pi / W)          # [kw, w]
    ICr = a[:, None] * np.cos(th_c) * s
    ICi = -a[:, None] * np.sin(th_c) * s
    f32 = lambda x: np.ascontiguousarray(x, dtype=np.float32)
    return {
        "FHRI": f32(np.concatenate([FHr_T[:, 0:65], FHi_T[:, 0:65]], axis=1)),  # [128,130]
        "FW_A": f32(np.concatenate([Fwr, Fwi], axis=1)),       # [128, 130]
        "FW_B": f32(np.concatenate([-Fwi, Fwr], axis=1)),      # [128, 130]
        "FW_C": f32(np.concatenate([Fwi, -Fwr], axis=1)),      # [128, 130]
        "IHr65": f32(IHr_T[:, 0:65]),                          # [128, 65]
        "IHi65": f32(IHi_T[:, 0:65]),                          # [128, 65]
        "ICr65": f32(ICr[:, 0:65]),                            # [65, 65]
        "ICi65": f32(ICi[:, 0:65]),                            # [65, 65]
        "IDENT": f32(np.eye(128)),                             # [128, 128]
    }


def build_nc(repeat=1, dummy_io=False):
    nc = bacc.Bacc(None, target_bir_lowering=False, debug=False)

    if dummy_io:
        nc.declare_dram_parameter("dummy_in", [1, 1], F32, isOutput=False)
        x_ext = nc.dram_tensor("x_int", [C, H, W], F32)
    else:
        x_ext = nc.declare_dram_parameter("x", [C, H, W], F32, isOutput=False)
    w1_ext = nc.declare_dram_parameter("w1", [2, G, D, D], F32, isOutput=False)
    w2_ext = nc.declare_dram_parameter("w2", [2, G, D, D], F32, isOutput=False)
    b1_ext = nc.declare_dram_parameter("b1", [2, G, D], F32, isOutput=False)
    b2_ext = nc.declare_dram_parameter("b2", [2, G, D], F32, isOutput=False)
    if dummy_io:
        nc.declare_dram_parameter("dummy_out", [1, 1], F32, isOutput=True)
        out_ext = nc.dram_tensor("out_int", [C, H, W], F32)
    else:
        out_ext = nc.declare_dram_parameter("out", [C, H, W], F32, isOutput=True)

    cdat = _consts()
    ckeys = list(cdat.keys())
    blob_cols = []
    coff = {}
    off = 0
    for kk in ckeys:
        v = cdat[kk]
        pad = np.zeros((128, v.shape[1]), np.float32)
        pad[0:v.shape[0], :] = v
        blob_cols.append(pad)
        coff[kk] = (off, v.shape[0], v.shape[1])
        off += v.shape[1]
    cblob = np.concatenate(blob_cols, axis=1)
    cblob_dram = nc.inline_tensor(np.ascontiguousarray(cblob), "c_blob")

    with tile.TileContext(nc) as tc:
        with (
            tc.tile_pool(name="cst", bufs=1) as cst,
            tc.tile_pool(name="big", bufs=1) as big,
            tc.tile_pool(name="work", bufs=1) as work,
            tc.tile_pool(name="psum", bufs=8, space="PSUM") as psp,
        ):
            # ---- constants: DMA fp32 staging, convert to bf16 ----
            cstg = work.tile([128, 1024], F32, tag="stg0", name="cstg")[:, 0:cblob.shape[1]]
            nc.scalar.dma_start(cstg[:], cblob_dram[:])

            cbf = {}
            for kk in ckeys:
                o, p, w = coff[kk]
                t = cst.tile([p, w], BF16, name=f"bf_{kk}")
                nc.vector.tensor_copy(t[:], cstg[0:p, o:o + w])
                cbf[kk] = t

            # ---- weights/biases for all 4 groups ----
            w1stg = work.tile([128, 1024], F32, tag="stg0", name="w1stg")
            w2stg = work.tile([128, 1024], F32, tag="stg0", name="w2stg")
            nc.scalar.dma_start(w1stg[:].rearrange("i (t g o) -> i t g o", t=2, g=G),
                                w1_ext[:].transpose([2, 0, 1, 3]))
            nc.scalar.dma_start(w2stg[:].rearrange("i (t g o) -> i t g o", t=2, g=G),
                                w2_ext[:].transpose([2, 0, 1, 3]))
            b1stg = cst.tile([128, 8], F32, name="b1stg")
            b2stg = cst.tile([128, 8], F32, name="b2stg")
            nc.scalar.dma_start(b1stg[:].rearrange("p (t g) -> p t g", t=2),
                                b1_ext[:].transpose([2, 0, 1]))
            nc.scalar.dma_start(b2stg[:].rearrange("p (t g) -> p t g", t=2),
                                b2_ext[:].transpose([2, 0, 1]))
            wts = []
            for g in range(G):
                d = {}
                for nm, stg, comp in [
                    ("w1r", w1stg, 0), ("w1i", w1stg, 1),
                    ("w2r", w2stg, 0), ("w2i", w2stg, 1),
                ]:
                    wsl = stg[:, comp * 512 + g * 128: comp * 512 + (g + 1) * 128]
                    wbf = cst.tile([128, 128], BF16, name=f"w_{g}_{nm}")
                    nc.vector.tensor_copy(wbf[:], wsl)
                    d[nm] = wbf
                    if nm in ("w1i", "w2i"):
                        wneg = cst.tile([128, 128], BF16, name=f"w_{g}_{nm}n")
                        nc.vector.tensor_scalar_mul(wneg[:], wsl, -1.0)
                        d[nm + "n"] = wneg
                d["b1r"] = b1stg[:, g:g + 1]
                d["b1i"] = b1stg[:, 4 + g:4 + g + 1]
                d["b2r"] = b2stg[:, g:g + 1]
                d["b2i"] = b2stg[:, 4 + g:4 + g + 1]
                wts.append(d)

            FHRI = cbf["FHRI"]
            IHr65, IHi65 = cbf["IHr65"], cbf["IHi65"]
            ICr65, ICi65 = cbf["ICr65"], cbf["ICi65"]
            IDENT = cbf["IDENT"]

            for rep in range(repeat):
              drainq = []

              def drain(n):
                  for _ in range(min(n, len(drainq))):
                      drainq.pop(0)()

              for g in range(G):
                wt = wts[g]
                c0 = g * D

                # ============ PHASE A: x load/convert + S1 ============
                # 4-channel x chunks: 32 per group
                xr_tiles = []
                for i in range(32):
                    xf = work.tile([128, 512], F32, tag="xf", bufs=2,
                                   name=f"xf_{rep}_{g}_{i}", uniquify=False)
                    nc.sync.dma_start(
                        xf[:].rearrange("p (c w) -> p c w", w=128),
                        x_ext[c0 + 4 * i:c0 + 4 * i + 4].transpose([1, 0, 2]))
                    xr = work.tile([128, 512], BF16, tag="xr", bufs=3,
                                   name=f"xr_{rep}_{g}_{i}", uniquify=False)
                    nc.gpsimd.tensor_copy(xr[:], xf[:])
                    xr_tiles.append(xr)
                    drain(1)

                def xch(c):  # lhsT [h, w] for channel index c (0..127)
                    return xr_tiles[c // 4][:, (c % 4) * 128:(c % 4) * 128 + 128]

                # S1: per ch: psum[w, 130] = x_c^T @ FHRI; 6 ch per psum tile
                # Y layout: [w, c*130 + t*65 + k]
                Y = big.tile([128, D * 130], BF16, tag="Y", bufs=1, name=f"Y_{rep}_{g}")
                ti = 0
                ch0 = 0
                while ch0 < D:
                    n = min(6, D - ch0)
                    ps = psp.tile([128, 1024], F32, tag="psA", bufs=2,
                                  name=f"s1p_{rep}_{g}_{ch0}")
                    for j in range(n):
                        o = (j % 3) * 130 + (j // 3) * 512
                        nc.tensor.matmul(ps[:, o:o + 130], xch(ch0 + j), FHRI[:],
                                         start=True, stop=True)
                    if n == 6:
                        srcv = ps[:].rearrange("p (b x) -> p b x", b=2)[:, :, 0:390]
                        dv = Y[:, ch0 * 130:(ch0 + 6) * 130].rearrange(
                            "p (b x) -> p b x", b=2)
                    else:
                        srcv = ps[:, 0:n * 130]
                        dv = Y[:, ch0 * 130:(ch0 + n) * 130]
                    if ti % 2 == 0:
                        nc.scalar.copy(dv, srcv)
                    else:
                        nc.vector.tensor_copy(dv, srcv)
                    drain(1)
                    ch0 += n
                    ti += 1

                # ============ PHASE B: S2 + S3 + S4/gate ============
                SrSi = big.tile([128, 2 * NF], BF16, tag="S", bufs=1,
                                name=f"S_{rep}_{g}")
                Hr = big.tile([128, NF], BF16, tag="Hr", bufs=1, name=f"Hr_{rep}_{g}")
                Hi = big.tile([128, NF], BF16, tag="Hi", bufs=1, name=f"Hi_{rep}_{g}")
                Gr = big.tile([128, NF], BF16, tag="Gr", bufs=1, name=f"Gr_{rep}_{g}")
                Gi = big.tile([128, NF], BF16, tag="Gi", bufs=1, name=f"Gi_{rep}_{g}")
                Yv = Y[:].rearrange("p (c t k) -> p c t k", t=2, k=65)

                orr = oi = None
                s3k = 0
                s4k = 0
                N_S3 = (NF + 511) // 512  # 17

                def emit_s3(k):
                    f0 = 512 * k
                    n = min(512, NF - f0)
                    ps = psp.tile([128, 1024], F32, tag="psA", bufs=2,
                                  name=f"s3p_{rep}_{g}_{k}")
                    pr, pi = ps[:, 0:512], ps[:, 512:1024]
                    sr_c = SrSi[:, f0:f0 + n]
                    si_c = SrSi[:, NF + f0:NF + f0 + n]
                    nc.tensor.matmul(pr[:, :n], wt["w1r"][:], sr_c, start=True, stop=False)
                    nc.tensor.matmul(pr[:, :n], wt["w1in"][:], si_c, start=False, stop=True)
                    nc.tensor.matmul(pi[:, :n], wt["w1i"][:], sr_c, start=True, stop=False)
                    nc.tensor.matmul(pi[:, :n], wt["w1r"][:], si_c, start=False, stop=True)
                    nc.scalar.activation(Hr[:, f0:f0 + n], pr[:, :n], AF.Relu,
                                         bias=wt["b1r"])
                    nc.scalar.activation(Hi[:, f0:f0 + n], pi[:, :n], AF.Relu,
                                         bias=wt["b1i"])

                def emit_s4(k):
                    nonlocal orr, oi
                    f0 = 512 * k
                    n = min(512, NF - f0)
                    if k % 2 == 0:
                        orr = work.tile([128, 1024], BF16, tag="orr", bufs=2,
                                        name=f"or_{rep}_{g}_{k}")
                        oi = work.tile([128, 1024], BF16, tag="oib", bufs=2,
                                       name=f"oi_{rep}_{g}_{k}")
                    lo = (k % 2) * 512
                    ps = psp.tile([128, 1024], F32, tag="psA", bufs=2,
                                  name=f"s4p_{rep}_{g}_{k}")
                    por, poi = ps[:, 0:512], ps[:, 512:1024]
                    hr_c, hi_c = Hr[:, f0:f0 + n], Hi[:, f0:f0 + n]
                    nc.tensor.matmul(por[:, :n], wt["w2r"][:], hr_c, start=True, stop=False)
                    nc.tensor.matmul(por[:, :n], wt["w2in"][:], hi_c, start=False, stop=True)
                    nc.tensor.matmul(poi[:, :n], wt["w2i"][:], hr_c, start=True, stop=False)
                    nc.tensor.matmul(poi[:, :n], wt["w2r"][:], hi_c, start=False, stop=True)
                    nc.scalar.activation(orr[:, lo:lo + n], por[:, :n], AF.Identity,
                                         bias=wt["b2r"])
                    nc.scalar.activation(oi[:, lo:lo + n], poi[:, :n], AF.Identity,
                                         bias=wt["b2i"])
                    if k % 2 == 1 or k == N_S3 - 1:
                        b0 = f0 - lo
                        bn = lo + n
                        sr_b = SrSi[:, b0:b0 + bn]
                        si_b = SrSi[:, NF + b0:NF + b0 + bn]
                        t2 = work.tile([128, 1024], BF16, tag="t2", bufs=1,
                                       name=f"t2_{rep}_{g}_{k}", uniquify=False)
                        t4 = work.tile([128, 1024], BF16, tag="t2", bufs=1,
                                       name=f"t4_{rep}_{g}_{k}", uniquify=False)
                        ob, ib = orr[:, 0:bn], oi[:, 0:bn]
                        nc.vector.tensor_mul(Gr[:, b0:b0 + bn], ob, sr_b)
                        nc.vector.tensor_mul(t2[:, 0:bn], ib, si_b)
                        nc.vector.tensor_sub(Gr[:, b0:b0 + bn], Gr[:, b0:b0 + bn],
                                             t2[:, 0:bn])
                        nc.vector.tensor_mul(Gi[:, b0:b0 + bn], ob, si_b)
                        nc.vector.tensor_mul(t4[:, 0:bn], ib, sr_b)
                        nc.vector.tensor_add(Gi[:, b0:b0 + bn], Gi[:, b0:b0 + bn],
                                             t4[:, 0:bn])

                # S2 tiles: 6 kh each; kh-major S layout
                kh0 = 0
                ti = 0
                while kh0 < KH:
                    nk = min(6, KH - kh0)
                    ps = psp.tile([128, 1024], F32, tag="psA", bufs=2,
                                  name=f"s2p_{rep}_{g}_{kh0}")
                    for j in range(nk):
                        kh = kh0 + j
                        idx = kh if kh <= 64 else 128 - kh
                        rhsB = cbf["FW_B"] if kh <= 64 else cbf["FW_C"]
                        o = (j % 3) * 130 + (j // 3) * 512
                        op = ps[:, o:o + 130]
                        nc.tensor.matmul(op, Yv[:, :, 0, idx], cbf["FW_A"][:],
                                         start=True, stop=False)
                        nc.tensor.matmul(op, Yv[:, :, 1, idx], rhsB[:],
                                         start=False, stop=True)
                    # evac: one instr per component t
                    for t in range(2):
                        if nk == 6:
                            srcv = ps[:].rearrange("p (b x) -> p b x", b=2)[
                                :, :, 0:390].rearrange(
                                "p b (k s) -> p b k s", k=3)[:, :, :, t * 65:(t + 1) * 65]
                            dv = SrSi[:, t * NF + kh0 * 65: t * NF + (kh0 + 6) * 65]
                            dv = dv.rearrange("p (b k s) -> p b k s", b=2, k=3)
                        else:
                            nb = min(3, nk)
                            srcv = ps[:, 0:nb * 130].rearrange(
                                "p (k s) -> p k s", k=nb)[:, :, t * 65:(t + 1) * 65]
                            dv = SrSi[:, t * NF + kh0 * 65: t * NF + (kh0 + nb) * 65]
                            dv = dv.rearrange("p (k s) -> p k s", k=nb)
                        nc.scalar.copy(dv, srcv)
                    kh0 += nk
                    ti += 1
                    drain(1)
                    # S3 chunk k ready when kh coverage reaches its f-range
                    while s3k < N_S3 and kh0 * 65 >= min(512 * (s3k + 1), NF):
                        emit_s3(s3k)
                        s3k += 1
                        if s4k < s3k - 1:
                            emit_s4(s4k)
                            s4k += 1
                        drain(1)
                while s3k < N_S3:
                    emit_s3(s3k)
                    s3k += 1
                    drain(1)
                while s4k < N_S3:
                    emit_s4(s4k)
                    s4k += 1
                    drain(1)

                # ============ PHASE C: S5 transposes ============
                GT = big.tile([128, 2 * NF], BF16, tag="GT", bufs=1,
                              name=f"GT_{rep}_{g}")
                for t, Gsrc in ((0, Gr), (1, Gi)):
                    Gv = Gsrc[:].rearrange("p (k w) -> p k w", w=65)
                    for bt in range(5):  # 13 kw per tile
                        kw0 = 13 * bt
                        ps = psp.tile([128, 1024], F32, tag="psA", bufs=2,
                                      name=f"s5p_{rep}_{g}_{t}_{bt}")
                        p16 = ps[:].bitcast(BF16)
                        for j in range(13):
                            nc.tensor.transpose(p16[:, j * 128:(j + 1) * 128],
                                                Gv[:, :, kw0 + j], IDENT[:])
                        nc.vector.tensor_copy(
                            GT[:, t * NF + kw0 * 128: t * NF + (kw0 + 13) * 128],
                            p16[:, 0:13 * 128])
                        drain(1)

                # ============ PHASE D: queue S6/S7 batches for draining ============
                GTv = GT[:].rearrange("p (t k c) -> p t k c", t=2, c=128)

                ytile = [None]

                def mk_s6(b, g=g, GTv=GTv):
                    def emit():
                        cb = 4 * b
                        ps = psp.tile([128, 2048], F32, tag="psC", bufs=1,
                                      name=f"s6p_{rep}_{g}_{b}")
                        for j in range(4):
                            c = cb + j
                            gtr = GTv[:, 0, :, c]
                            gti = GTv[:, 1, :, c]
                            o = j * 65
                            nc.tensor.matmul(ps[0:65, o:o + 65], gtr, IHr65[:],
                                             start=True, stop=True)          # M1
                            nc.tensor.matmul(ps[0:65, 1024 + o:1024 + o + 65], gtr,
                                             IHi65[:], start=True, stop=True)  # M3
                            nc.tensor.matmul(ps[0:65, 512 + o:512 + o + 65], gti,
                                             IHi65[:], start=True, stop=True)  # M2
                            nc.tensor.matmul(ps[0:65, 1536 + o:1536 + o + 65], gti,
                                             IHr65[:], start=True, stop=True)  # M4
                        vt = work.tile([65, 1024], BF16, tag="V", bufs=3,
                                       name=f"v_{rep}_{g}_{b}", uniquify=False)
                        vv = vt[:].rearrange("p (j t h) -> p j t h", j=4, t=2)
                        m = [ps[0:65, q * 512:q * 512 + 260].rearrange(
                            "p (j h) -> p j h", j=4) for q in range(4)]
                        mr = [ps[0:65, q * 512:q * 512 + 260].rearrange(
                            "p (j h) -> p j h", j=4)[:, :, 63:0:-1] for q in range(4)]
                        # Vr-lo = M1 - M2 ; Vr-hi = rev(M1 + M2)
                        nc.vector.tensor_sub(vv[:, :, 0, 0:65], m[0], m[1])
                        nc.gpsimd.tensor_add(vv[:, :, 0, 65:128], mr[0], mr[1])
                        # Vi-lo = M3 + M4 ; Vi-hi = rev(M4 - M3)
                        nc.vector.tensor_add(vv[:, :, 1, 0:65], m[2], m[3])
                        nc.gpsimd.tensor_sub(vv[:, :, 1, 65:128], mr[3], mr[2])
                        return vt
                    return emit

                def mk_s7(b, vt_holder, g=g, c0=c0, ytile=ytile):
                    def emit():
                        cb = 4 * b
                        vt = vt_holder[0]
                        ps = psp.tile([128, 1024], F32, tag="psA", bufs=2,
                                      name=f"s7p_{rep}_{g}_{b}")
                        for j in range(4):
                            o = j * 256
                            vr = vt[:, j * 256:j * 256 + 128]
                            vi = vt[:, j * 256 + 128:j * 256 + 256]
                            nc.tensor.matmul(ps[:, o:o + 65], vr, ICr65[:],
                                             start=True, stop=True)          # N1
                            nc.tensor.matmul(ps[:, o + 65:o + 130], vi, ICi65[:],
                                             start=True, stop=True)          # N2
                        if b % 2 == 0:
                            ytile[0] = work.tile([128, 1024], F32, tag="y", bufs=2,
                                                 name=f"y_{rep}_{g}_{b}",
                                                 uniquify=False)
                        yt = ytile[0]
                        yo = (b % 2) * 512
                        yv = yt[:, yo:yo + 512].rearrange("p (j w) -> p j w", j=4)
                        n1 = ps[:].rearrange("p (j x) -> p j x", j=4)
                        # y-lo = N1 + N2 ; y-hi = rev(N1 - N2)
                        nc.vector.tensor_add(yv[:, :, 0:65],
                                             n1[:, :, 0:65], n1[:, :, 65:130])
                        nc.gpsimd.tensor_sub(yv[:, :, 65:128],
                                             n1[:, :, 63:0:-1],
                                             n1[:, :, 128:65:-1])
                        if b % 2 == 1:
                            nc.sync.dma_start(
                                out_ext[c0 + 4 * (b - 1):c0 + 4 * (b - 1) + 8
                                        ].transpose([1, 0, 2]),
                                yt[:].rearrange("p (c w) -> p c w", w=128))
                    return emit

                vh = {}
                for b in range(32):
                    vh[b] = [None]

                    def mk6(b=b):
                        def emit():
                            vh[b][0] = mk_s6(b)()
                        return emit
                    drainq.append(mk6())
                    if b >= 1:
                        drainq.append(mk_s7(b - 1, vh[b - 1]))
                drainq.append(mk_s7(31, vh[31]))

              # final drain (last group's S6/S7)
              drain(10 ** 9)

    nc.compile()
    return nc


_NC = None


def _get_nc():
    global _NC
    if _NC is None:
        _NC = build_nc()
    return _NC


def kernel(x, w1, w2, b1, b2, trace=False):
    nc = _get_nc()
    x = np.ascontiguousarray(x, dtype=np.float32)
    ins = {
        "w1": np.ascontiguousarray(w1, dtype=np.float32),
        "w2": np.ascontiguousarray(w2, dtype=np.float32),
        "b1": np.ascontiguousarray(b1, dtype=np.float32),
        "b2": np.ascontiguousarray(b2, dtype=np.float32),
    }
    in_maps = [dict(ins, x=x[i]) for i in range(NCORES)]
    res = run_bass_kernel_spmd(nc, in_maps, list(range(NCORES)), trace=trace)
    out = np.stack([np.asarray(r["out"], dtype=np.float32) for r in res.results])
    if trace:
        return out, res
    return out
